# revision 1
# baseline (speedup 1.0000x reference)
"""nn_DMSAttentionWrapper kernel for Trainium2, 8 NeuronCores.

The reference's eviction/causal mask is `jnp.maximum(dms, causal)` where the
two -inf regions are disjoint (dms: q > k+WIN and evicted; causal: q < k), so
the combined additive mask is identically zero: the oracle computes *dense,
non-causal, unmasked* multi-head attention.  The decision head (Wd, bd) does
not affect the output at all.

Sharding: data-parallel over batch (2) x tensor-parallel over heads (4 groups
of 4 heads).  Each core, for its (batch b, head-group g):
  - projects Qt/Kt (head-dim x T layout) and V (T x 4*HD) from x^T; everything
    bf16 with fp32 PSUM accumulation, fully SBUF-resident (no DRAM spills),
  - S^T = K Q^T per (k-tile, q-chunk); exp on ScalarE (no max subtraction:
    |scores| < ~6); denominator accumulated on PE via a ones-column matmul,
  - O^T = V^T E / denom (reciprocal on DVE, partition-broadcast on GpSimd),
  - partial = O_heads @ Wo[row-slice]; host sums 4 partials per batch.
The kt loop is software-pipelined (S leads PV/denom by 3) and each q-chunk's
Wo block is deferred by one chunk so PE never waits on the normalization
chain.  SCALE is folded into Wq on the host.
"""

import numpy as np
from contextlib import ExitStack

import ml_dtypes
import concourse.bass as bass
import concourse.tile as tile
from concourse import bacc, mybir
from concourse.bass_utils import run_bass_kernel_spmd

B, T, D, H = 2, 2048, 2048, 16
HD = 128
NCORES = 8
CPB = NCORES // B          # cores per batch
HPC = H // CPB             # heads per core
HS = HPC * HD              # head-slice width (columns of Wq/Wk/Wv, rows of Wo)
SCALE = 1.0 / float(np.sqrt(HD))

F32 = mybir.dt.float32
BF16 = mybir.dt.bfloat16

P = 128                    # partition dim
NF = 512                   # matmul free dim / psum bank
DT = D // P                # 16 contraction tiles over D
QC = T // NF               # 4 q chunks
KT = T // P                # 16 k tiles
TT = T // P                # 16 t tiles
LEAD = 2                   # S-matmul lead over PV/denom in the kt pipeline

ALU = mybir.AluOpType
ACTF = mybir.ActivationFunctionType

_CACHE: dict = {}


def _build():
    if "nc" in _CACHE:
        return _CACHE["nc"]

    nc = bacc.Bacc("TRN2", target_bir_lowering=False, debug=False)

    xT = nc.dram_tensor("xT", [D, T], BF16, kind="ExternalInput").ap()
    wq_d = nc.dram_tensor("wq", [D, HS], BF16, kind="ExternalInput").ap()
    wk_d = nc.dram_tensor("wk", [D, HS], BF16, kind="ExternalInput").ap()
    wv_d = nc.dram_tensor("wv", [D, HS], BF16, kind="ExternalInput").ap()
    wo_d = nc.dram_tensor("wo", [HS, D], BF16, kind="ExternalInput").ap()
    out_d = nc.dram_tensor("out", [T, D], F32, kind="ExternalOutput").ap()

    with tile.TileContext(nc) as tc, ExitStack() as ctx:
        const_pool = ctx.enter_context(tc.tile_pool(name="const", bufs=1))
        qk_pool = ctx.enter_context(tc.tile_pool(name="qk", bufs=1))
        v_pool = ctx.enter_context(tc.tile_pool(name="v", bufs=1))

        ones_f32 = const_pool.tile([P, 1], F32, name="ones_f32")
        nc.gpsimd.memset(ones_f32[:], 1.0)
        ones_col = const_pool.tile([P, 1], BF16, name="ones_col")
        nc.vector.tensor_copy(ones_col[:], ones_f32[:])
        # resident Q/K (head-dim x T, bf16) and V (T x 4*HD, bf16)
        qall = [qk_pool.tile([P, T], BF16, name=f"qh{h}") for h in range(HPC)]
        kall = [qk_pool.tile([P, T], BF16, name=f"kh{h}") for h in range(HPC)]
        vall = [v_pool.tile([P, HS], BF16, name=f"vt{i}") for i in range(KT)]

        # =================== Phase A: projections ===================
        with ExitStack() as actx:
            xt_pool = actx.enter_context(tc.tile_pool(name="xt", bufs=1))
            w_pool = actx.enter_context(tc.tile_pool(name="w", bufs=1))
            ps_a = actx.enter_context(tc.tile_pool(name="ps_a", bufs=2, space="PSUM"))

            # DMA order matters: wq first so the first projection matmul can
            # start ~8us in, instead of queueing behind all 16 xT tiles.
            wq_t, wk_t, wv_t = [], [], []
            for i in range(DT):
                t = w_pool.tile([P, HS], BF16, name=f"wq{i}")
                nc.sync.dma_start(t[:], wq_d[i * P:(i + 1) * P, :])
                wq_t.append(t)
            xt = []
            for i in range(DT):
                t = xt_pool.tile([P, T], BF16, name=f"xt{i}")
                nc.sync.dma_start(t[:], xT[i * P:(i + 1) * P, :])
                xt.append(t)
            for i in range(DT):
                t = w_pool.tile([P, HS], BF16, name=f"wk{i}")
                nc.sync.dma_start(t[:], wk_d[i * P:(i + 1) * P, :])
                wk_t.append(t)
            for i in range(DT):
                t = w_pool.tile([P, HS], BF16, name=f"wv{i}")
                nc.sync.dma_start(t[:], wv_d[i * P:(i + 1) * P, :])
                wv_t.append(t)

            # Q/K projections, qc-inner so each LDWEIGHTS serves 4 matmuls
            for wt, dsts in ((wq_t, qall), (wk_t, kall)):
                for h in range(HPC):
                    pss = [ps_a.tile([P, NF], F32, name="ps", tag=f"ps{qc}")
                           for qc in range(QC)]
                    for dt_i in range(DT):
                        for qc in range(QC):
                            nc.tensor.matmul(
                                pss[qc][:], wt[dt_i][:, h * HD:(h + 1) * HD],
                                xt[dt_i][:, qc * NF:(qc + 1) * NF],
                                start=(dt_i == 0), stop=(dt_i == DT - 1),
                            )
                    for qc in range(QC):
                        nc.vector.tensor_copy(
                            dsts[h][:, qc * NF:(qc + 1) * NF], pss[qc][:])

            for tt_i in range(TT):
                ps = ps_a.tile([P, NF], F32, name="ps", tag="ps0")
                for dt_i in range(DT):
                    nc.tensor.matmul(
                        ps[:], xt[dt_i][:, tt_i * P:(tt_i + 1) * P],
                        wv_t[dt_i][:],
                        start=(dt_i == 0), stop=(dt_i == DT - 1),
                    )
                nc.vector.tensor_copy(vall[tt_i][:], ps[:])

        # =================== Phase B: attention + Wo ===================
        with ExitStack() as bctx:
            e_pool = bctx.enter_context(tc.tile_pool(name="e", bufs=6))
            r_pool = bctx.enter_context(tc.tile_pool(name="r", bufs=2))
            ot_pool = bctx.enter_context(tc.tile_pool(name="ot", bufs=1))
            wo_pool = bctx.enter_context(tc.tile_pool(name="wo", bufs=1))
            obounce = bctx.enter_context(tc.tile_pool(name="obounce", bufs=4))
            ps_s = bctx.enter_context(tc.tile_pool(name="ps_s", bufs=3, space="PSUM"))
            ps_o = bctx.enter_context(tc.tile_pool(name="ps_o", bufs=3, space="PSUM"))
            ps_d = bctx.enter_context(tc.tile_pool(name="ps_d", bufs=2, space="PSUM"))

            wo_sb = []
            for h in range(HPC):
                t = wo_pool.tile([P, D], BF16, name=f"wo{h}")
                nc.sync.dma_start(t[:], wo_d[h * HD:(h + 1) * HD, :])
                wo_sb.append(t)

            ot = [ot_pool.tile([P, T], BF16, name=f"ot{h}") for h in range(HPC)]

            def wo_block(qc):
                for tt_i in range(4 * qc, 4 * (qc + 1)):
                    for dc in range(QC):
                        pw = ps_o.tile([P, NF], F32, name="po")
                        for h in range(HPC):
                            nc.tensor.matmul(
                                pw[:], ot[h][:, tt_i * P:(tt_i + 1) * P],
                                wo_sb[h][:, dc * NF:(dc + 1) * NF],
                                start=(h == 0), stop=(h == HPC - 1),
                            )
                        ob = obounce.tile([P, NF], F32, name="ob")
                        nc.vector.tensor_copy(ob[:], pw[:])
                        nc.sync.dma_start(
                            out_d[tt_i * P:(tt_i + 1) * P,
                                  dc * NF:(dc + 1) * NF], ob[:])

            for qc in range(QC):
                for h in range(HPC):
                    qh, kh = qall[h], kall[h]
                    po = ps_o.tile([P, NF], F32, name="po")
                    pd = ps_d.tile([1, NF], F32, name="pd")
                    es = {}
                    # software-pipelined kt loop: S leads PV/denom by LEAD
                    for step in range(KT + LEAD):
                        kt_i = step
                        if kt_i < KT:
                            ps = ps_s.tile([P, NF], F32, name="ps_st")
                            nc.tensor.matmul(
                                ps[:], kh[:, kt_i * P:(kt_i + 1) * P],
                                qh[:, qc * NF:(qc + 1) * NF],
                                start=True, stop=True,
                            )
                            e = e_pool.tile([P, NF], BF16, name="e")
                            nc.scalar.activation(e[:], ps[:], ACTF.Exp)
                            es[kt_i] = e
                        kt_j = step - LEAD
                        if kt_j >= 0:
                            e = es.pop(kt_j)
                            nc.tensor.matmul(
                                po[:], vall[kt_j][:, h * HD:(h + 1) * HD], e[:],
                                start=(kt_j == 0), stop=(kt_j == KT - 1),
                            )
                            nc.tensor.matmul(
                                pd[:], ones_col[:], e[:],
                                start=(kt_j == 0), stop=(kt_j == KT - 1),
                            )
                    recip = r_pool.tile([1, NF], F32, name="recip")
                    nc.vector.reciprocal(recip[:], pd[:])
                    rb = r_pool.tile([P, NF], F32, name="rb")
                    nc.gpsimd.partition_broadcast(rb[:], recip[:])
                    nc.vector.tensor_mul(
                        ot[h][:, qc * NF:(qc + 1) * NF], po[:], rb[:])
                if qc > 0:
                    wo_block(qc - 1)
            wo_block(QC - 1)

    nc.compile()
    _CACHE["nc"] = nc
    return nc


def make_in_maps(hidden_states, Wq, Wk, Wv, Wo, **kwargs):
    bf = ml_dtypes.bfloat16
    hidden_states = np.asarray(hidden_states, np.float32)
    # fold the attention scale into Wq so no scaling is needed on-device
    Wq = np.asarray(Wq, np.float32) * SCALE
    Wk, Wv, Wo = (np.asarray(a, np.float32) for a in (Wk, Wv, Wo))
    in_maps = []
    for c in range(NCORES):
        b, g = divmod(c, CPB)
        cols = slice(g * HS, (g + 1) * HS)
        in_maps.append(dict(
            xT=np.ascontiguousarray(hidden_states[b].T).astype(bf),
            wq=np.ascontiguousarray(Wq[:, cols]).astype(bf),
            wk=np.ascontiguousarray(Wk[:, cols]).astype(bf),
            wv=np.ascontiguousarray(Wv[:, cols]).astype(bf),
            wo=np.ascontiguousarray(Wo[cols, :]).astype(bf),
        ))
    return in_maps


def gather(results):
    out = np.zeros((B, T, D), np.float32)
    for c in range(NCORES):
        out[c // CPB] += results[c]["out"]
    return out


def kernel(hidden_states, Wq, Wk, Wv, Wo, Wd=None, bd=None, **kwargs):
    nc = _build()
    in_maps = make_in_maps(hidden_states, Wq, Wk, Wv, Wo)
    res = run_bass_kernel_spmd(nc, in_maps, core_ids=list(range(NCORES)))
    return gather(res.results)



# revision 4
# speedup vs baseline: 1.4563x; 1.4563x over previous
"""nn_DMSAttentionWrapper kernel for Trainium2, 8 NeuronCores.

The reference's eviction/causal mask is `jnp.maximum(dms, causal)` where the
two -inf regions are disjoint (dms: q > k+WIN and evicted; causal: q < k), so
the combined additive mask is identically zero: the oracle computes *dense,
non-causal, unmasked* multi-head attention.  The decision head (Wd, bd) does
not affect the output at all.

Sharding: data-parallel over batch (2) x tensor-parallel over heads (4 groups
of 4 heads).  Each core, for its (batch b, head-group g):
  - projects Qt/Kt (head-dim x T layout) and V (T x 4*HD) from x^T; everything
    bf16 with fp32 PSUM accumulation, fully SBUF-resident,
  - attention per (q-chunk of 1024, head): S^T into a 2-bank PSUM tile
    (2 matmuls), ONE exp ACTIVATE over [128,1024] (amortizes ScalarE
    per-instruction overhead), PV accumulated on PE,
  - softmax denominator WITHOUT per-kt ones-matmuls: exp tiles accumulated on
    DVE (bf16 2x mode), then a single ones[128,128] matmul both
    partition-reduces and broadcasts the denominator; reciprocal via
    reciprocal_approx_fast on DVE (no gpsimd broadcast, no slow iterative
    reciprocal),
  - partial = O_heads @ Wo[row-slice] interleaved at group boundaries so PE
    fills the normalization-chain latency; host sums 4 bf16 partials/batch.
SCALE is folded into Wq on the host.
"""

import numpy as np
from contextlib import ExitStack

import ml_dtypes
import concourse.bass as bass
import concourse.tile as tile
from concourse import bacc, mybir
from concourse.bass_utils import run_bass_kernel_spmd

B, T, D, H = 2, 2048, 2048, 16
HD = 128
NCORES = 8
CPB = NCORES // B          # cores per batch
HPC = H // CPB             # heads per core
HS = HPC * HD              # head-slice width (columns of Wq/Wk/Wv, rows of Wo)
SCALE = 1.0 / float(np.sqrt(HD))

F32 = mybir.dt.float32
BF16 = mybir.dt.bfloat16

P = 128                    # partition dim
NF = 512                   # matmul free dim / psum bank (fp32)
QH = 1024                  # q macro-chunk (exp ACTIVATE free dim, 2 psum banks)
NQH = T // QH              # 2 q macro-chunks
DT = D // P                # 16 contraction tiles over D
KT = T // P                # 16 k tiles
TT = T // P                # 16 t tiles
LEAD = 2                   # S/exp lead over PV/denom in the kt pipeline

ALU = mybir.AluOpType
ACTF = mybir.ActivationFunctionType

_CACHE: dict = {}


def _build():
    if "nc" in _CACHE:
        return _CACHE["nc"]

    nc = bacc.Bacc("TRN2", target_bir_lowering=False, debug=False)

    xT = nc.dram_tensor("xT", [D, T], BF16, kind="ExternalInput").ap()
    wq_d = nc.dram_tensor("wq", [D, HS], BF16, kind="ExternalInput").ap()
    wk_d = nc.dram_tensor("wk", [D, HS], BF16, kind="ExternalInput").ap()
    wv_d = nc.dram_tensor("wv", [D, HS], BF16, kind="ExternalInput").ap()
    wo_d = nc.dram_tensor("wo", [HS, D], BF16, kind="ExternalInput").ap()
    out_d = nc.dram_tensor("out", [T, D], BF16, kind="ExternalOutput").ap()

    with tile.TileContext(nc) as tc, ExitStack() as ctx:
        const_pool = ctx.enter_context(tc.tile_pool(name="const", bufs=1))
        qk_pool = ctx.enter_context(tc.tile_pool(name="qk", bufs=1))
        v_pool = ctx.enter_context(tc.tile_pool(name="v", bufs=1))
        wo_pool = ctx.enter_context(tc.tile_pool(name="wo", bufs=1))

        ones_f32 = const_pool.tile([P, P], F32, name="ones_f32")
        nc.gpsimd.memset(ones_f32[:], 1.0)
        ones128 = const_pool.tile([P, P], BF16, name="ones128")
        nc.vector.tensor_copy(ones128[:], ones_f32[:])

        # resident Q/K (head-dim x T, bf16) and V (T x 4*HD, bf16)
        qall = [qk_pool.tile([P, T], BF16, name=f"qh{h}") for h in range(HPC)]
        kall = [qk_pool.tile([P, T], BF16, name=f"kh{h}") for h in range(HPC)]
        vall = [v_pool.tile([P, HS], BF16, name=f"vt{i}") for i in range(KT)]

        # =================== Phase A: projections ===================
        with ExitStack() as actx:
            xt_pool = actx.enter_context(tc.tile_pool(name="xt", bufs=1))
            w_pool = actx.enter_context(tc.tile_pool(name="w", bufs=1))
            ps_a = actx.enter_context(tc.tile_pool(name="ps_a", bufs=2, space="PSUM"))
            ps_v = actx.enter_context(tc.tile_pool(name="ps_v", bufs=2, space="PSUM"))

            # DMA order matters: wq first so the first projection matmul can
            # start as soon as possible.
            wq_t, wk_t, wv_t = [], [], []
            for i in range(DT):
                t = w_pool.tile([P, HS], BF16, name=f"wq{i}")
                nc.sync.dma_start(t[:], wq_d[i * P:(i + 1) * P, :])
                wq_t.append(t)
            xt = []
            for i in range(DT):
                t = xt_pool.tile([P, T], BF16, name=f"xt{i}")
                nc.sync.dma_start(t[:], xT[i * P:(i + 1) * P, :])
                xt.append(t)
            for i in range(DT):
                t = w_pool.tile([P, HS], BF16, name=f"wk{i}")
                nc.sync.dma_start(t[:], wk_d[i * P:(i + 1) * P, :])
                wk_t.append(t)
            for i in range(DT):
                t = w_pool.tile([P, HS], BF16, name=f"wv{i}")
                nc.sync.dma_start(t[:], wv_d[i * P:(i + 1) * P, :])
                wv_t.append(t)
            wo_sb = []
            for h in range(HPC):
                t = wo_pool.tile([P, D], BF16, name=f"wo{h}")
                nc.sync.dma_start(t[:], wo_d[h * HD:(h + 1) * HD, :])
                wo_sb.append(t)

            # Q/K projections: per (h, q-macro-chunk), one [128,1024] psum
            for wt, dsts in ((wq_t, qall), (wk_t, kall)):
                for h in range(HPC):
                    for qh in range(NQH):
                        ps = ps_a.tile([P, QH], F32, name="psqk")
                        for dt_i in range(DT):
                            for hf in range(2):
                                nc.tensor.matmul(
                                    ps[:, hf * NF:(hf + 1) * NF],
                                    wt[dt_i][:, h * HD:(h + 1) * HD],
                                    xt[dt_i][:, qh * QH + hf * NF:
                                             qh * QH + (hf + 1) * NF],
                                    start=(dt_i == 0), stop=(dt_i == DT - 1),
                                )
                        nc.vector.tensor_copy(
                            dsts[h][:, qh * QH:(qh + 1) * QH], ps[:])

            for tt_i in range(TT):
                ps = ps_v.tile([P, NF], F32, name="psv")
                for dt_i in range(DT):
                    nc.tensor.matmul(
                        ps[:], xt[dt_i][:, tt_i * P:(tt_i + 1) * P],
                        wv_t[dt_i][:],
                        start=(dt_i == 0), stop=(dt_i == DT - 1),
                    )
                nc.vector.tensor_copy(vall[tt_i][:], ps[:])

        # =================== Phase B: attention + Wo ===================
        with ExitStack() as bctx:
            e_pool = bctx.enter_context(tc.tile_pool(name="e", bufs=4))
            es_pool = bctx.enter_context(tc.tile_pool(name="es", bufs=2))
            r_pool = bctx.enter_context(tc.tile_pool(name="r", bufs=3))
            ot_pool = bctx.enter_context(tc.tile_pool(name="ot", bufs=1))
            obounce = bctx.enter_context(tc.tile_pool(name="obounce", bufs=4))
            ps_s = bctx.enter_context(tc.tile_pool(name="ps_s", bufs=2, space="PSUM"))
            ps_o = bctx.enter_context(tc.tile_pool(name="ps_o", bufs=1, space="PSUM"))
            ps_w = bctx.enter_context(tc.tile_pool(name="ps_w", bufs=2, space="PSUM"))

            ot = [ot_pool.tile([P, T], BF16, name=f"ot{h}") for h in range(HPC)]

            def wo_slice(qhb, tts):
                # out partial for ot columns tts (within q-block qhb), all dc
                for tt_i in tts:
                    for dc in range(T // NF):
                        pw = ps_w.tile([P, NF], F32, name="pw")
                        for h in range(HPC):
                            nc.tensor.matmul(
                                pw[:], ot[h][:, tt_i * P:(tt_i + 1) * P],
                                wo_sb[h][:, dc * NF:(dc + 1) * NF],
                                start=(h == 0), stop=(h == HPC - 1),
                            )
                        ob = obounce.tile([P, NF], BF16, name="ob")
                        nc.vector.tensor_copy(ob[:], pw[:])
                        nc.sync.dma_start(
                            out_d[tt_i * P:(tt_i + 1) * P,
                                  dc * NF:(dc + 1) * NF], ob[:])

            groups = [(qh, h) for qh in range(NQH) for h in range(HPC)]
            for gi, (qh, h) in enumerate(groups):
                qs = qh * QH
                po = ps_o.tile([P, QH], F32, name="po")
                esum = es_pool.tile([P, QH], BF16, name="esum")
                es = {}
                # software-pipelined kt loop: S/exp leads PV/denom by LEAD
                for step in range(KT + LEAD):
                    kt_i = step
                    if kt_i < KT:
                        ps = ps_s.tile([P, QH], F32, name="ps_st", tag="s")
                        for hf in range(2):
                            nc.tensor.matmul(
                                ps[:, hf * NF:(hf + 1) * NF],
                                kall[h][:, kt_i * P:(kt_i + 1) * P],
                                qall[h][:, qs + hf * NF:qs + (hf + 1) * NF],
                                start=True, stop=True,
                            )
                        e = e_pool.tile([P, QH], BF16, name="e")
                        nc.scalar.activation(e[:], ps[:], ACTF.Exp)
                        es[kt_i] = e
                    kt_j = step - LEAD
                    if kt_j >= 0:
                        e = es.pop(kt_j)
                        for hf in range(2):
                            nc.tensor.matmul(
                                po[:, hf * NF:(hf + 1) * NF],
                                vall[kt_j][:, h * HD:(h + 1) * HD],
                                e[:, hf * NF:(hf + 1) * NF],
                                start=(kt_j == 0), stop=(kt_j == KT - 1),
                            )
                        if kt_j == 0:
                            nc.vector.tensor_copy(esum[:], e[:])
                        else:
                            nc.vector.tensor_add(esum[:], esum[:], e[:])
                # denominator: ones^T @ esum reduces over k AND broadcasts to
                # all 128 partitions in one shot
                pd = ps_s.tile([P, QH], F32, name="pd", tag="s")
                for hf in range(2):
                    nc.tensor.matmul(
                        pd[:, hf * NF:(hf + 1) * NF], ones128[:],
                        esum[:, hf * NF:(hf + 1) * NF],
                        start=True, stop=True,
                    )
                dsb = r_pool.tile([P, QH], F32, name="dsb")
                nc.vector.tensor_copy(dsb[:], pd[:])
                rb = r_pool.tile([P, QH], F32, name="rb")
                nc.vector.reciprocal_approx_fast(rb[:], dsb[:])
                nc.vector.tensor_mul(ot[h][:, qs:qs + QH], po[:], rb[:])
                # interleave Wo of the previous q-block to fill the
                # normalization-chain latency at group boundaries
                if gi >= HPC:
                    bi = gi - HPC
                    wo_slice(0, range(2 * bi, 2 * bi + 2))
            wo_slice(1, range(TT // 2, TT))

    nc.compile()
    _CACHE["nc"] = nc
    return nc


def make_in_maps(hidden_states, Wq, Wk, Wv, Wo, **kwargs):
    bf = ml_dtypes.bfloat16
    hidden_states = np.asarray(hidden_states, np.float32)
    # fold the attention scale into Wq so no scaling is needed on-device
    Wq = np.asarray(Wq, np.float32) * SCALE
    Wk, Wv, Wo = (np.asarray(a, np.float32) for a in (Wk, Wv, Wo))
    in_maps = []
    for c in range(NCORES):
        b, g = divmod(c, CPB)
        cols = slice(g * HS, (g + 1) * HS)
        in_maps.append(dict(
            xT=np.ascontiguousarray(hidden_states[b].T).astype(bf),
            wq=np.ascontiguousarray(Wq[:, cols]).astype(bf),
            wk=np.ascontiguousarray(Wk[:, cols]).astype(bf),
            wv=np.ascontiguousarray(Wv[:, cols]).astype(bf),
            wo=np.ascontiguousarray(Wo[cols, :]).astype(bf),
        ))
    return in_maps


def gather(results):
    out = np.zeros((B, T, D), np.float32)
    for c in range(NCORES):
        out[c // CPB] += np.asarray(results[c]["out"], np.float32)
    return out


def kernel(hidden_states, Wq, Wk, Wv, Wo, Wd=None, bd=None, **kwargs):
    nc = _build()
    in_maps = make_in_maps(hidden_states, Wq, Wk, Wv, Wo)
    res = run_bass_kernel_spmd(nc, in_maps, core_ids=list(range(NCORES)))
    return gather(res.results)
